# revision 8
# baseline (speedup 1.0000x reference)
"""Trainium2 Bass kernel for nn_DilatedResSkipBlock.

Reference math (per batch element b):
    w      = weight_norm(conv_v, conv_g)                  # [256, 128, 3]
    h      = causal_dilated_conv(x, w, dil=2, pad_left=4) + conv_b
    a, bb  = split(h, 2)                                  # [128, T] each
    c      = lc_w @ condition                             # [256, T]
    ca, cb = split(c, 2)
    g      = tanh(a + ca) * sigmoid(bb + cb)              # [128, T]
    s      = skip_w @ g + skip_b
    o      = out_w @ g + out_b + x
    return (o, s)

Sharding: data-parallel over batch -- 8 batch elements, one per NeuronCore.
Each core processes its full [128, 32768] time axis, so the dilated conv
needs no cross-core halo exchange.

Per-core kernel: time axis tiled at 2048 cols per DMA tile, 512 cols per
PSUM subtile.  All tensors move and compute in bf16 (fp32 accumulation in
PSUM); inputs are converted to bf16 on the host, which halves input DMA
traffic vs fp32.  All channel mixing runs on the tensor engine:
    a_psum = sum_k WaT_k @ x[t+2k-4] + lcaT @ cond     (4 matmuls)
    b_psum = sum_k WbT_k @ x[t+2k-4] + lcbT @ cond     (4 matmuls)
    ta     = tanh(a_psum + ba)            (ScalarE, bf16 out)
    tb     = tanh(0.5*b_psum + bb)        (ScalarE, bf16 out)
    g      = ta*(1+tb) = 2*g_true         (VectorE stt, bf16 2x mode)
    s_out  = skipT/2 @ g + skip_b         (1 matmul + ScalarE bias add)
    o_out  = outT/2 @ g + out_b + x       (1 matmul + VectorE stt)
sigmoid(z) = (1 + tanh(z/2))/2, so the b-half activation runs Tanh with
scale=0.5 -- ACT then only ever uses the Tanh table (no table reloads).
The trailing 1/2 is folded into halved skip/out weights.
"""

import numpy as np

RES, GATE, K, DIL, CIN = 128, 256, 3, 2, 80
PAD = (K - 1) * DIL  # 4
B, T = 8, 32768
N_CORES = 8
TILE = 2048   # columns per DMA tile
SUB = 512     # columns per PSUM subtile (one PSUM bank of fp32)
N_TILES = T // TILE
N_SUB = TILE // SUB

# wts packing layout (single [128, 1280] bf16 dram input):
#   cols 0:768     conv lhsT, 6 blocks of 128: block (h*3+k) = w[h*128:(h+1)*128, :, k].T
#   cols 768:896   lc_a lhsT  (rows 0:80 valid)
#   cols 896:1024  lc_b lhsT  (rows 0:80 valid)
#   cols 1024:1152 skip lhsT / 2
#   cols 1152:1280 out lhsT / 2
# bias [128, 4] fp32 dram input:
#   col 0 conv_b[:128], col 1 conv_b[128:]/2 (sigmoid-as-tanh bias),
#   col 2 skip_b, col 3 out_b
WTS_COLS = 1280

_CACHE = {}


def _build_nc(reps=1, unroll_reps=None):
    import contextlib

    import concourse.bacc as bacc
    import concourse.tile as tile
    from concourse import mybir

    f32 = mybir.dt.float32
    bf16 = mybir.dt.bfloat16
    Act = mybir.ActivationFunctionType
    Alu = mybir.AluOpType

    nc = bacc.Bacc("TRN2", target_bir_lowering=False, debug=False,
                   num_devices=N_CORES)

    x_d = nc.dram_tensor("x", [RES, T], bf16, kind="ExternalInput").ap()
    c_d = nc.dram_tensor("condition", [CIN, T], bf16, kind="ExternalInput").ap()
    w_d = nc.dram_tensor("wts", [128, WTS_COLS], bf16, kind="ExternalInput").ap()
    bias_d = nc.dram_tensor("bias", [128, 4], f32, kind="ExternalInput").ap()
    z_d = nc.dram_tensor("zpad", [128, PAD], bf16, kind="ExternalInput").ap()
    o_d = nc.dram_tensor("o", [RES, T], bf16, kind="ExternalOutput").ap()
    s_d = nc.dram_tensor("s", [RES, T], bf16, kind="ExternalOutput").ap()

    with tile.TileContext(nc) as tc:
        with (
            tc.tile_pool(name="wpool", bufs=1) as wpool,
            tc.tile_pool(name="io", bufs=4) as io,
            tc.tile_pool(name="work", bufs=3) as work,
            tc.tile_pool(name="psum", bufs=1, space="PSUM") as psum,
        ):
            wts = wpool.tile([128, WTS_COLS], bf16)
            nc.sync.dma_start(wts[:], w_d[:])
            biases = wpool.tile([128, 4], f32)
            nc.sync.dma_start(biases[:], bias_d[:])

            def conv_lhsT(h, k):
                c0 = (h * 3 + k) * 128
                return wts[:, c0:c0 + 128]

            lc_lhsT = [wts[0:CIN, 768:896], wts[0:CIN, 896:1024]]
            skip_lhsT = wts[:, 1024:1152]
            out_lhsT = wts[:, 1152:1280]
            bias_a = biases[:, 0:1]
            bias_b = biases[:, 1:2]
            skip_b = biases[:, 2:3]
            out_b = biases[:, 3:4]

            n_unroll = unroll_reps if unroll_reps is not None else 1
            rep_loop = (tc.For_i(0, reps, 1) if reps > 1
                        else contextlib.nullcontext())

            PAIR = 2 * SUB

            def emit_C(pend):
                """Lagged skip/out matmuls + store epilogue for one pair."""
                (g2, x_t, o_t, s_t, sub0, tile_i) = pend
                lo0 = sub0 * SUB
                s_ps = psum.tile([128, PAIR], f32, tag="s", name="s")
                o_ps = psum.tile([128, PAIR], f32, tag="o", name="o")
                for q in range(2):
                    nc.tensor.matmul(s_ps[:, q * SUB:(q + 1) * SUB], skip_lhsT,
                                     g2[:, q * SUB:(q + 1) * SUB],
                                     start=True, stop=True)
                for q in range(2):
                    nc.tensor.matmul(o_ps[:, q * SUB:(q + 1) * SUB], out_lhsT,
                                     g2[:, q * SUB:(q + 1) * SUB],
                                     start=True, stop=True)
                nc.scalar.activation(s_t[:, lo0:lo0 + PAIR], s_ps[:],
                                     Act.Identity, bias=skip_b)
                nc.vector.scalar_tensor_tensor(
                    o_t[:, lo0:lo0 + PAIR], o_ps[:], out_b,
                    x_t[:, PAD + lo0:PAD + lo0 + PAIR],
                    op0=Alu.add, op1=Alu.add)
                if sub0 == 2:  # second pair of the tile: tile fully stored
                    t0 = tile_i * TILE
                    nc.sync.dma_start(o_d[:, t0:t0 + TILE], o_t[:])
                    nc.sync.dma_start(s_d[:, t0:t0 + TILE], s_t[:])

            with rep_loop:
                for _rep in range(n_unroll):
                    pend = None
                    x_t = c_t = o_t = s_t = None
                    for j in range(N_TILES * 2):
                        i, p = divmod(j, 2)
                        if p == 0:
                            t0 = i * TILE
                            x_t = io.tile([RES, TILE + PAD], bf16, tag="x")
                            if i == 0:
                                nc.sync.dma_start(x_t[:, 0:PAD], z_d[:])
                                nc.sync.dma_start(x_t[:, PAD:],
                                                  x_d[:, 0:TILE])
                            else:
                                nc.sync.dma_start(
                                    x_t[:], x_d[:, t0 - PAD:t0 + TILE])
                            c_t = io.tile([CIN, TILE], bf16, tag="cond")
                            nc.sync.dma_start(c_t[:], c_d[:, t0:t0 + TILE])
                            o_t = io.tile([RES, TILE], bf16, tag="o")
                            s_t = io.tile([RES, TILE], bf16, tag="s")

                        sub0 = p * 2
                        lo0 = sub0 * SUB
                        # conv+lc matmuls, weight-major over the 2 subtiles
                        # (lc first so psum completes early for the ACT chain)
                        b_ps = psum.tile([128, PAIR], f32, tag="b", name="b")
                        a_ps = psum.tile([128, PAIR], f32, tag="a", name="a")
                        for ps2, lcw, h in ((b_ps, lc_lhsT[1], 1),
                                            (a_ps, lc_lhsT[0], 0)):
                            for q in range(2):
                                lo = (sub0 + q) * SUB
                                nc.tensor.matmul(
                                    ps2[:, q * SUB:(q + 1) * SUB], lcw,
                                    c_t[:, lo:lo + SUB],
                                    start=True, stop=False)
                            for k in range(K):
                                for q in range(2):
                                    lo = (sub0 + q) * SUB + DIL * k
                                    nc.tensor.matmul(
                                        ps2[:, q * SUB:(q + 1) * SUB],
                                        conv_lhsT(h, k),
                                        x_t[:, lo:lo + SUB],
                                        start=False, stop=(k == K - 1))

                        ta = work.tile([128, PAIR], bf16, tag="ta", name="ta")
                        tb = work.tile([128, PAIR], bf16, tag="tb", name="tb")
                        g2 = work.tile([128, PAIR], bf16, tag="g", name="g")
                        nc.scalar.activation(tb[:], b_ps[:], Act.Tanh,
                                             bias=bias_b, scale=0.5)
                        nc.scalar.activation(ta[:], a_ps[:], Act.Tanh,
                                             bias=bias_a)
                        nc.vector.scalar_tensor_tensor(
                            g2[:], tb[:], 1.0, ta[:],
                            op0=Alu.add, op1=Alu.mult)

                        if pend is not None:
                            emit_C(pend)
                        pend = (g2, x_t, o_t, s_t, sub0, i)
                    emit_C(pend)

    nc.compile()
    return nc


def _get_nc(reps=1):
    key = ("nc", reps)
    if key not in _CACHE:
        _CACHE[key] = _build_nc(reps)
    return _CACHE[key]


def _pack_wts(conv_v, conv_g, conv_b, lc_v, lc_g, skip_v, skip_g, skip_b,
              out_v, out_g, out_b):
    import ml_dtypes

    def wn(v, g):
        norm = np.sqrt(np.sum(v * v, axis=(1, 2), keepdims=True))
        return v * (g.reshape(-1, 1, 1) / norm)

    conv_w = wn(conv_v, conv_g)            # [256, 128, 3]
    lc_w = wn(lc_v, lc_g)[:, :, 0]         # [256, 80]
    skip_w = wn(skip_v, skip_g)[:, :, 0]   # [128, 128]
    out_w = wn(out_v, out_g)[:, :, 0]      # [128, 128]

    wts = np.zeros((128, WTS_COLS), np.float32)
    for h in range(2):
        for k in range(K):
            c0 = (h * 3 + k) * 128
            wts[:, c0:c0 + 128] = conv_w[h * 128:(h + 1) * 128, :, k].T
    wts[0:CIN, 768:896] = lc_w[0:128].T
    wts[0:CIN, 896:1024] = lc_w[128:256].T
    wts[:, 1024:1152] = skip_w.T * 0.5
    wts[:, 1152:1280] = out_w.T * 0.5

    bias = np.zeros((128, 4), np.float32)
    bias[:, 0] = conv_b[0:128]
    bias[:, 1] = conv_b[128:256] * 0.5
    bias[:, 2] = skip_b
    bias[:, 3] = out_b
    return wts.astype(ml_dtypes.bfloat16), bias


def _prep_arrays(inputs):
    import ml_dtypes

    bf16 = ml_dtypes.bfloat16
    f = lambda k: np.asarray(inputs[k], dtype=np.float32)
    x = np.ascontiguousarray(f("x").astype(bf16))
    cond = np.ascontiguousarray(f("condition").astype(bf16))
    wts, bias = _pack_wts(f("conv_v"), f("conv_g"), f("conv_b"), f("lc_v"),
                          f("lc_g"), f("skip_v"), f("skip_g"), f("skip_b"),
                          f("out_v"), f("out_g"), f("out_b"))
    zpad = np.zeros((128, PAD), bf16)
    return x, cond, wts, bias, zpad


def run(inputs, trace=False, **trace_kwargs):
    from concourse.bass_utils import run_bass_kernel_spmd

    x, cond, wts, bias, zpad = _prep_arrays(inputs)

    nc = _get_nc()
    in_maps = [{"x": x[b], "condition": cond[b], "wts": wts, "bias": bias,
                "zpad": zpad}
               for b in range(N_CORES)]
    res = run_bass_kernel_spmd(nc, in_maps, list(range(N_CORES)),
                               trace=trace, **trace_kwargs)
    o = np.stack([res.results[b]["o"] for b in range(N_CORES)]).astype(np.float32)
    s = np.stack([res.results[b]["s"] for b in range(N_CORES)]).astype(np.float32)
    return (o, s), res


def kernel(**inputs):
    out, _ = run(inputs, trace=False)
    return out


def _make_device_runner(nc):
    """jit-compiled 8-core runner with device-resident inputs (no donation,
    no per-call host transfer) for wall-clock timing."""
    import jax
    import numpy as np
    from jax.experimental.shard_map import shard_map
    from jax.sharding import Mesh, NamedSharding, PartitionSpec

    from concourse import mybir
    from concourse.bass2jax import (_bass_exec_p, install_neuronx_cc_hook,
                                    partition_id_tensor)

    install_neuronx_cc_hook()
    partition_name = (nc.partition_id_tensor.name
                      if nc.partition_id_tensor else None)
    in_names, out_names, out_avals, zero_outs = [], [], [], []
    for alloc in nc.m.functions[0].allocations:
        if not isinstance(alloc, mybir.MemoryLocationSet):
            continue
        name = alloc.memorylocations[0].name
        if alloc.kind == "ExternalInput":
            if name != partition_name:
                in_names.append(name)
        elif alloc.kind == "ExternalOutput":
            shape = tuple(alloc.tensor_shape)
            dtype = mybir.dt.np(alloc.dtype)
            out_names.append(name)
            out_avals.append(jax.core.ShapedArray(shape, dtype))
            zero_outs.append(np.zeros(shape, dtype))
    n_params = len(in_names)
    all_in_names = list(in_names) + list(out_names)
    if partition_name is not None:
        all_in_names.append(partition_name)

    def _body(*args):
        operands = list(args)
        if partition_name is not None:
            operands.append(partition_id_tensor())
        return tuple(_bass_exec_p.bind(
            *operands,
            out_avals=tuple(out_avals),
            in_names=tuple(all_in_names),
            out_names=tuple(out_names),
            lowering_input_output_aliases=(),
            sim_require_finite=True,
            sim_require_nnan=True,
            nc=nc,
        ))

    devices = jax.devices()[:N_CORES]
    mesh = Mesh(np.asarray(devices), ("core",))
    spec = PartitionSpec("core")
    f = jax.jit(shard_map(_body, mesh=mesh,
                          in_specs=(spec,) * (n_params + len(out_names)),
                          out_specs=(spec,) * len(out_names),
                          check_rep=False),
                keep_unused=True)

    def put(per_core_arrays):
        # per_core_arrays: list over inputs of list over cores
        sharding = NamedSharding(mesh, spec)
        out = []
        for arrs in per_core_arrays:
            out.append(jax.device_put(
                np.concatenate(arrs, axis=0), sharding))
        return out

    return f, put, in_names, n_params, zero_outs


def measure_exec_ns(inputs, reps=512, iters=8):
    """Estimate per-invocation HW time via (wall[reps] - wall[1]) / (reps-1)
    with device-resident inputs; host/dispatch overhead cancels in the delta."""
    import statistics
    import time

    import jax

    x, cond, wts, bias, zpad = _prep_arrays(inputs)
    data = {"x": x, "condition": cond,
            "wts": np.broadcast_to(wts, (N_CORES,) + wts.shape),
            "bias": np.broadcast_to(bias, (N_CORES,) + bias.shape),
            "zpad": np.broadcast_to(zpad, (N_CORES,) + zpad.shape)}

    def bench(nc):
        fjit, put, in_names, n_params, zero_outs = _make_device_runner(nc)
        per_core = [[data[n][b] for b in range(N_CORES)] for n in in_names]
        per_core += [[z for _ in range(N_CORES)] for z in zero_outs]
        dev_args = put(per_core)
        r = fjit(*dev_args)
        jax.block_until_ready(r)  # compile + warm
        ts = []
        for _ in range(iters):
            t0 = time.perf_counter()
            r = fjit(*dev_args)
            jax.block_until_ready(r)
            ts.append(time.perf_counter() - t0)
        return ts

    t1 = bench(_get_nc(1))
    tr = bench(_get_nc(reps))
    fmt = lambda ts: "[" + " ".join(f"{t * 1e3:.1f}" for t in ts) + "] ms"
    print(f"  wall[1]    {fmt(t1)}")
    print(f"  wall[{reps}] {fmt(tr)}")
    w1, wr = statistics.median(t1), statistics.median(tr)
    ns = (wr - w1) / (reps - 1) * 1e9
    nsmin = (min(tr) - min(t1)) / (reps - 1) * 1e9
    print(f"  median delta {ns:.0f} ns/iter, min delta {nsmin:.0f} ns/iter")
    return ns


# revision 9
# speedup vs baseline: 1.0207x; 1.0207x over previous
"""Trainium2 Bass kernel for nn_DilatedResSkipBlock.

Reference math (per batch element b):
    w      = weight_norm(conv_v, conv_g)                  # [256, 128, 3]
    h      = causal_dilated_conv(x, w, dil=2, pad_left=4) + conv_b
    a, bb  = split(h, 2)                                  # [128, T] each
    c      = lc_w @ condition                             # [256, T]
    ca, cb = split(c, 2)
    g      = tanh(a + ca) * sigmoid(bb + cb)              # [128, T]
    s      = skip_w @ g + skip_b
    o      = out_w @ g + out_b + x
    return (o, s)

Sharding: data-parallel over batch -- 8 batch elements, one per NeuronCore.
Each core processes its full [128, 32768] time axis, so the dilated conv
needs no cross-core halo exchange.

Per-core kernel: time axis tiled at 2048 cols per DMA tile, 512 cols per
PSUM subtile.  All tensors move and compute in bf16 (fp32 accumulation in
PSUM); inputs are converted to bf16 on the host, which halves input DMA
traffic vs fp32.  All channel mixing runs on the tensor engine:
    a_psum = sum_k WaT_k @ x[t+2k-4] + lcaT @ cond     (4 matmuls)
    b_psum = sum_k WbT_k @ x[t+2k-4] + lcbT @ cond     (4 matmuls)
    ta     = tanh(a_psum + ba)            (ScalarE, bf16 out)
    tb     = tanh(0.5*b_psum + bb)        (ScalarE, bf16 out)
    g      = ta*(1+tb) = 2*g_true         (VectorE stt, bf16 2x mode)
    s_out  = skipT/2 @ g + skip_b         (1 matmul + ScalarE bias add)
    o_out  = outT/2 @ g + out_b + x       (1 matmul + VectorE stt)
sigmoid(z) = (1 + tanh(z/2))/2, so the b-half activation runs Tanh with
scale=0.5 -- ACT then only ever uses the Tanh table (no table reloads).
The trailing 1/2 is folded into halved skip/out weights.
"""

import numpy as np

RES, GATE, K, DIL, CIN = 128, 256, 3, 2, 80
PAD = (K - 1) * DIL  # 4
B, T = 8, 32768
N_CORES = 8
TILE = 2048   # columns per DMA tile
SUB = 512     # columns per PSUM subtile (one PSUM bank of fp32)
N_TILES = T // TILE
N_SUB = TILE // SUB

# wts packing layout (single [128, 1280] bf16 dram input):
#   cols 0:768     conv lhsT, 6 blocks of 128: block (h*3+k) = w[h*128:(h+1)*128, :, k].T
#   cols 768:896   lc_a lhsT  (rows 0:80 valid)
#   cols 896:1024  lc_b lhsT  (rows 0:80 valid)
#   cols 1024:1152 skip lhsT / 2
#   cols 1152:1280 out lhsT / 2
# bias [128, 4] fp32 dram input:
#   col 0 conv_b[:128], col 1 conv_b[128:]/2 (sigmoid-as-tanh bias),
#   col 2 skip_b, col 3 out_b
WTS_COLS = 1280

_CACHE = {}


def _build_nc(reps=1, unroll_reps=None):
    import contextlib

    import concourse.bacc as bacc
    import concourse.tile as tile
    from concourse import mybir

    f32 = mybir.dt.float32
    bf16 = mybir.dt.bfloat16
    Act = mybir.ActivationFunctionType
    Alu = mybir.AluOpType

    nc = bacc.Bacc("TRN2", target_bir_lowering=False, debug=False,
                   num_devices=N_CORES)

    x_d = nc.dram_tensor("x", [RES, T], bf16, kind="ExternalInput").ap()
    c_d = nc.dram_tensor("condition", [CIN, T], bf16, kind="ExternalInput").ap()
    w_d = nc.dram_tensor("wts", [128, WTS_COLS], bf16, kind="ExternalInput").ap()
    bias_d = nc.dram_tensor("bias", [128, 4], f32, kind="ExternalInput").ap()
    z_d = nc.dram_tensor("zpad", [128, PAD], bf16, kind="ExternalInput").ap()
    o_d = nc.dram_tensor("o", [RES, T], bf16, kind="ExternalOutput").ap()
    s_d = nc.dram_tensor("s", [RES, T], bf16, kind="ExternalOutput").ap()

    with tile.TileContext(nc) as tc:
        with (
            tc.tile_pool(name="wpool", bufs=1) as wpool,
            tc.tile_pool(name="io", bufs=4) as io,
            tc.tile_pool(name="work", bufs=6) as work,
            tc.tile_pool(name="psum", bufs=2, space="PSUM") as psum,
        ):
            wts = wpool.tile([128, WTS_COLS], bf16)
            nc.sync.dma_start(wts[:], w_d[:])
            biases = wpool.tile([128, 4], f32)
            nc.sync.dma_start(biases[:], bias_d[:])

            def conv_lhsT(h, k):
                c0 = (h * 3 + k) * 128
                return wts[:, c0:c0 + 128]

            lc_lhsT = [wts[0:CIN, 768:896], wts[0:CIN, 896:1024]]
            skip_lhsT = wts[:, 1024:1152]
            out_lhsT = wts[:, 1152:1280]
            bias_a = biases[:, 0:1]
            bias_b = biases[:, 1:2]
            skip_b = biases[:, 2:3]
            out_b = biases[:, 3:4]

            n_unroll = unroll_reps if unroll_reps is not None else 1
            rep_loop = (tc.For_i(0, reps, 1) if reps > 1
                        else contextlib.nullcontext())

            def emit_C(pend):
                """Lagged skip/out matmuls + store epilogue for one pair."""
                (g2, x_t, o_t, s_t, sub0, tile_i) = pend
                s_ps = [psum.tile([128, SUB], f32, tag="s", name=f"s{q}")
                        for q in range(2)]
                o_ps = [psum.tile([128, SUB], f32, tag="o", name=f"o{q}")
                        for q in range(2)]
                for q in range(2):
                    nc.tensor.matmul(s_ps[q][:], skip_lhsT, g2[q][:],
                                     start=True, stop=True)
                for q in range(2):
                    nc.tensor.matmul(o_ps[q][:], out_lhsT, g2[q][:],
                                     start=True, stop=True)
                for q in range(2):
                    lo = (sub0 + q) * SUB
                    nc.scalar.activation(s_t[:, lo:lo + SUB], s_ps[q][:],
                                         Act.Identity, bias=skip_b)
                    nc.vector.scalar_tensor_tensor(
                        o_t[:, lo:lo + SUB], o_ps[q][:], out_b,
                        x_t[:, PAD + lo:PAD + lo + SUB],
                        op0=Alu.add, op1=Alu.add)
                if sub0 == 2:  # second pair of the tile: tile fully stored
                    t0 = tile_i * TILE
                    nc.sync.dma_start(o_d[:, t0:t0 + TILE], o_t[:])
                    nc.sync.dma_start(s_d[:, t0:t0 + TILE], s_t[:])

            with rep_loop:
                for _rep in range(n_unroll):
                    pend = None
                    x_t = c_t = o_t = s_t = None
                    for j in range(N_TILES * 2):
                        i, p = divmod(j, 2)
                        if p == 0:
                            t0 = i * TILE
                            x_t = io.tile([RES, TILE + PAD], bf16, tag="x")
                            if i == 0:
                                nc.sync.dma_start(x_t[:, 0:PAD], z_d[:])
                                nc.sync.dma_start(x_t[:, PAD:],
                                                  x_d[:, 0:TILE])
                            else:
                                nc.sync.dma_start(
                                    x_t[:], x_d[:, t0 - PAD:t0 + TILE])
                            c_t = io.tile([CIN, TILE], bf16, tag="cond")
                            nc.sync.dma_start(c_t[:], c_d[:, t0:t0 + TILE])
                            o_t = io.tile([RES, TILE], bf16, tag="o")
                            s_t = io.tile([RES, TILE], bf16, tag="s")

                        sub0 = p * 2
                        # conv+lc matmuls, weight-major over the 2 subtiles
                        # (lc first so psum completes early for the ACT chain)
                        b_ps = [psum.tile([128, SUB], f32, tag="b",
                                          name=f"b{q}") for q in range(2)]
                        a_ps = [psum.tile([128, SUB], f32, tag="a",
                                          name=f"a{q}") for q in range(2)]
                        for ps2, lcw, h in ((b_ps, lc_lhsT[1], 1),
                                            (a_ps, lc_lhsT[0], 0)):
                            for q in range(2):
                                lo = (sub0 + q) * SUB
                                nc.tensor.matmul(
                                    ps2[q][:], lcw, c_t[:, lo:lo + SUB],
                                    start=True, stop=False)
                            for k in range(K):
                                for q in range(2):
                                    lo = (sub0 + q) * SUB + DIL * k
                                    nc.tensor.matmul(
                                        ps2[q][:], conv_lhsT(h, k),
                                        x_t[:, lo:lo + SUB],
                                        start=False, stop=(k == K - 1))

                        ta = [work.tile([128, SUB], bf16, tag="ta",
                                        name=f"ta{q}") for q in range(2)]
                        tb = [work.tile([128, SUB], bf16, tag="tb",
                                        name=f"tb{q}") for q in range(2)]
                        g2 = [work.tile([128, SUB], bf16, tag="g",
                                        name=f"g{q}") for q in range(2)]
                        for q in range(2):
                            nc.scalar.activation(tb[q][:], b_ps[q][:],
                                                 Act.Tanh, bias=bias_b,
                                                 scale=0.5)
                        for q in range(2):
                            nc.scalar.activation(ta[q][:], a_ps[q][:],
                                                 Act.Tanh, bias=bias_a)
                            nc.vector.scalar_tensor_tensor(
                                g2[q][:], tb[q][:], 1.0, ta[q][:],
                                op0=Alu.add, op1=Alu.mult)

                        if pend is not None:
                            emit_C(pend)
                        pend = (g2, x_t, o_t, s_t, sub0, i)
                    emit_C(pend)

    nc.compile()
    return nc


def _get_nc(reps=1):
    key = ("nc", reps)
    if key not in _CACHE:
        _CACHE[key] = _build_nc(reps)
    return _CACHE[key]


def _pack_wts(conv_v, conv_g, conv_b, lc_v, lc_g, skip_v, skip_g, skip_b,
              out_v, out_g, out_b):
    import ml_dtypes

    def wn(v, g):
        norm = np.sqrt(np.sum(v * v, axis=(1, 2), keepdims=True))
        return v * (g.reshape(-1, 1, 1) / norm)

    conv_w = wn(conv_v, conv_g)            # [256, 128, 3]
    lc_w = wn(lc_v, lc_g)[:, :, 0]         # [256, 80]
    skip_w = wn(skip_v, skip_g)[:, :, 0]   # [128, 128]
    out_w = wn(out_v, out_g)[:, :, 0]      # [128, 128]

    wts = np.zeros((128, WTS_COLS), np.float32)
    for h in range(2):
        for k in range(K):
            c0 = (h * 3 + k) * 128
            wts[:, c0:c0 + 128] = conv_w[h * 128:(h + 1) * 128, :, k].T
    wts[0:CIN, 768:896] = lc_w[0:128].T
    wts[0:CIN, 896:1024] = lc_w[128:256].T
    wts[:, 1024:1152] = skip_w.T * 0.5
    wts[:, 1152:1280] = out_w.T * 0.5

    bias = np.zeros((128, 4), np.float32)
    bias[:, 0] = conv_b[0:128]
    bias[:, 1] = conv_b[128:256] * 0.5
    bias[:, 2] = skip_b
    bias[:, 3] = out_b
    return wts.astype(ml_dtypes.bfloat16), bias


def _prep_arrays(inputs):
    import ml_dtypes

    bf16 = ml_dtypes.bfloat16
    f = lambda k: np.asarray(inputs[k], dtype=np.float32)
    x = np.ascontiguousarray(f("x").astype(bf16))
    cond = np.ascontiguousarray(f("condition").astype(bf16))
    wts, bias = _pack_wts(f("conv_v"), f("conv_g"), f("conv_b"), f("lc_v"),
                          f("lc_g"), f("skip_v"), f("skip_g"), f("skip_b"),
                          f("out_v"), f("out_g"), f("out_b"))
    zpad = np.zeros((128, PAD), bf16)
    return x, cond, wts, bias, zpad


def run(inputs, trace=False, **trace_kwargs):
    from concourse.bass_utils import run_bass_kernel_spmd

    x, cond, wts, bias, zpad = _prep_arrays(inputs)

    nc = _get_nc()
    in_maps = [{"x": x[b], "condition": cond[b], "wts": wts, "bias": bias,
                "zpad": zpad}
               for b in range(N_CORES)]
    res = run_bass_kernel_spmd(nc, in_maps, list(range(N_CORES)),
                               trace=trace, **trace_kwargs)
    o = np.stack([res.results[b]["o"] for b in range(N_CORES)]).astype(np.float32)
    s = np.stack([res.results[b]["s"] for b in range(N_CORES)]).astype(np.float32)
    return (o, s), res


def kernel(**inputs):
    out, _ = run(inputs, trace=False)
    return out


def _make_device_runner(nc):
    """jit-compiled 8-core runner with device-resident inputs (no donation,
    no per-call host transfer) for wall-clock timing."""
    import jax
    import numpy as np
    from jax.experimental.shard_map import shard_map
    from jax.sharding import Mesh, NamedSharding, PartitionSpec

    from concourse import mybir
    from concourse.bass2jax import (_bass_exec_p, install_neuronx_cc_hook,
                                    partition_id_tensor)

    install_neuronx_cc_hook()
    partition_name = (nc.partition_id_tensor.name
                      if nc.partition_id_tensor else None)
    in_names, out_names, out_avals, zero_outs = [], [], [], []
    for alloc in nc.m.functions[0].allocations:
        if not isinstance(alloc, mybir.MemoryLocationSet):
            continue
        name = alloc.memorylocations[0].name
        if alloc.kind == "ExternalInput":
            if name != partition_name:
                in_names.append(name)
        elif alloc.kind == "ExternalOutput":
            shape = tuple(alloc.tensor_shape)
            dtype = mybir.dt.np(alloc.dtype)
            out_names.append(name)
            out_avals.append(jax.core.ShapedArray(shape, dtype))
            zero_outs.append(np.zeros(shape, dtype))
    n_params = len(in_names)
    all_in_names = list(in_names) + list(out_names)
    if partition_name is not None:
        all_in_names.append(partition_name)

    def _body(*args):
        operands = list(args)
        if partition_name is not None:
            operands.append(partition_id_tensor())
        return tuple(_bass_exec_p.bind(
            *operands,
            out_avals=tuple(out_avals),
            in_names=tuple(all_in_names),
            out_names=tuple(out_names),
            lowering_input_output_aliases=(),
            sim_require_finite=True,
            sim_require_nnan=True,
            nc=nc,
        ))

    devices = jax.devices()[:N_CORES]
    mesh = Mesh(np.asarray(devices), ("core",))
    spec = PartitionSpec("core")
    f = jax.jit(shard_map(_body, mesh=mesh,
                          in_specs=(spec,) * (n_params + len(out_names)),
                          out_specs=(spec,) * len(out_names),
                          check_rep=False),
                keep_unused=True)

    def put(per_core_arrays):
        # per_core_arrays: list over inputs of list over cores
        sharding = NamedSharding(mesh, spec)
        out = []
        for arrs in per_core_arrays:
            out.append(jax.device_put(
                np.concatenate(arrs, axis=0), sharding))
        return out

    return f, put, in_names, n_params, zero_outs


def measure_exec_ns(inputs, reps=512, iters=8):
    """Estimate per-invocation HW time via (wall[reps] - wall[1]) / (reps-1)
    with device-resident inputs; host/dispatch overhead cancels in the delta."""
    import statistics
    import time

    import jax

    x, cond, wts, bias, zpad = _prep_arrays(inputs)
    data = {"x": x, "condition": cond,
            "wts": np.broadcast_to(wts, (N_CORES,) + wts.shape),
            "bias": np.broadcast_to(bias, (N_CORES,) + bias.shape),
            "zpad": np.broadcast_to(zpad, (N_CORES,) + zpad.shape)}

    def bench(nc):
        fjit, put, in_names, n_params, zero_outs = _make_device_runner(nc)
        per_core = [[data[n][b] for b in range(N_CORES)] for n in in_names]
        per_core += [[z for _ in range(N_CORES)] for z in zero_outs]
        dev_args = put(per_core)
        r = fjit(*dev_args)
        jax.block_until_ready(r)  # compile + warm
        ts = []
        for _ in range(iters):
            t0 = time.perf_counter()
            r = fjit(*dev_args)
            jax.block_until_ready(r)
            ts.append(time.perf_counter() - t0)
        return ts

    t1 = bench(_get_nc(1))
    tr = bench(_get_nc(reps))
    fmt = lambda ts: "[" + " ".join(f"{t * 1e3:.1f}" for t in ts) + "] ms"
    print(f"  wall[1]    {fmt(t1)}")
    print(f"  wall[{reps}] {fmt(tr)}")
    w1, wr = statistics.median(t1), statistics.median(tr)
    ns = (wr - w1) / (reps - 1) * 1e9
    nsmin = (min(tr) - min(t1)) / (reps - 1) * 1e9
    print(f"  median delta {ns:.0f} ns/iter, min delta {nsmin:.0f} ns/iter")
    return ns
